# revision 1
# baseline (speedup 1.0000x reference)
"""DeltaNet layer kernel for 8 Trainium2 NeuronCores.

Math note: in the reference's _delta_scan, the update added to the (D,D)
state h is identical for every row and h0=0, so all rows of h stay equal
forever. The layer therefore reduces exactly to a per-(batch, head)
first-order scalar-decay recurrence on a D-vector:

    c_t = beta_t * c_{t-1} + k_t * vsum_t,   o_t = qsum_t * c_t

with vsum = sum_d v, qsum = sum_d q. qsum/vsum only need x @ col-sums of
Wq/Wv. The recurrence maps 1:1 onto the DVE tensor_tensor_scan
instruction (fp32 state, one lane per (head, d) pair, scan along
tokens), which is bit-exact vs a sequential fp32 loop.

Two SPMD launches on cores 0-7:
  L1: core (b, head-group of 8): GEMM1 (f32r) -> extras rows
      (zbeta/qsum/vsum) + k rows; sigmoid; DRAM-bounce replication of
      beta/vs/qs rows to 64 lanes per head; u = k*vs fused into psum
      evacuation; tensor_tensor_scan; o = qs*c -> f32r.
  L2: core (b, token-half): GEMM2 o @ Wo (+ b_o via a k=1 matmul row),
      residual add, LayerNorm (ACT accum_out stats + fused
      tensor_scalar), * ln_g + ln_b.
"""
import sys

sys.path.insert(0, "/opt/trn_rl_repo")

import numpy as np

B, S, HID, NH = 4, 4096, 1024, 16
D = HID // NH
EPS = 1e-5
HG = 8          # heads per L1 core
TH = S // 2     # tokens per L2 core

_PROGRAMS = None


def _build_l1():
    import concourse.bass as bass
    import concourse.mybir as mybir
    from concourse import tile, bacc

    f32, f32r = mybir.dt.float32, mybir.dt.float32r
    AF = mybir.ActivationFunctionType
    ALU = mybir.AluOpType

    nc = bacc.Bacc("TRN2", target_bir_lowering=False, debug=False, num_devices=8)
    xT = nc.dram_tensor("xT", [HID, S], f32r, kind="ExternalInput")
    # Wcat columns: [zb(8) | qs(8) | vs(8) | k(512)]
    Wcat = nc.dram_tensor("Wcat", [HID, 536], f32r, kind="ExternalInput")
    bbeta = nc.dram_tensor("bbeta", [8, 1], f32, kind="ExternalInput")
    o_out = nc.dram_tensor("o_out", [HG * D, S], f32r, kind="ExternalOutput")

    KT = 8          # hid k-tiles
    NW = 512

    # selector matrices: exp[p, :] = fields_row(field, head(p)); head(p) = p // 64
    sels = []
    for mi in range(4):
        per_field = []
        for row0 in (0, 8, 16):  # beta/zb, qs, vs row blocks in extras
            m = np.zeros((24, 128), np.float32)
            for p in range(128):
                m[row0 + 2 * mi + p // 64, p] = 1.0
            per_field.append(m)
        sels.append(per_field)

    with tile.TileContext(nc) as tc:
        sel_dram = [[nc.inline_tensor(m, name=f"sel{mi}_{f}") for f, m in enumerate(row)]
                    for mi, row in enumerate(sels)]
        with tc.tile_pool(name="wc", bufs=1) as wc_pool, \
             tc.tile_pool(name="xt", bufs=2) as xt_pool, \
             tc.tile_pool(name="ksb", bufs=3) as ksb_pool, \
             tc.tile_pool(name="ext", bufs=2) as ext_pool, \
             tc.tile_pool(name="exp", bufs=3) as exp_pool, \
             tc.tile_pool(name="work", bufs=2) as work_pool, \
             tc.tile_pool(name="state", bufs=1) as state_pool, \
             tc.tile_pool(name="osb", bufs=3) as o_pool, \
             tc.tile_pool(name="ps", bufs=4, space="PSUM") as ps_pool, \
             tc.tile_pool(name="pse", bufs=1, space="PSUM") as pse_pool, \
             tc.tile_pool(name="psel", bufs=3, space="PSUM") as psel_pool:

            wc = wc_pool.tile([128, KT, 536], f32r)
            for k in range(KT):
                nc.sync.dma_start(out=wc[:, k, :], in_=Wcat[k * 128:(k + 1) * 128, :])
            bb = wc_pool.tile([8, 1], f32)
            nc.sync.dma_start(out=bb[:], in_=bbeta[:])
            selt = []
            for mi in range(4):
                per_field = []
                for f in range(3):
                    stf = wc_pool.tile([24, 128], f32, name=f"seltf{mi}_{f}")
                    nc.sync.dma_start(out=stf[:], in_=sel_dram[mi][f][:])
                    st = wc_pool.tile([24, 128], f32r, name=f"selt{mi}_{f}")
                    nc.vector.tensor_copy(st[:], stf[:])
                    per_field.append(st)
                selt.append(per_field)
            prev_c = [None] * 4

            BLK = 1024
            NB = BLK // NW

            def gemm_slab(blk, nn, extras, ksb):
                t0 = blk * BLK
                n0 = nn * NW
                xt = xt_pool.tile([128, KT, NW], f32r, tag="xt", name=f"xt{blk}_{nn}")
                nc.sync.dma_start(
                    out=xt[:],
                    in_=xT.rearrange("(kt p) s -> p kt s", p=128)[:, :, t0 + n0:t0 + n0 + NW])
                pse = pse_pool.tile([24, NW], f32, tag="pse", name=f"pse{blk}_{nn}")
                for k in range(KT):
                    nc.tensor.matmul(pse[:], wc[:, k, 0:24], xt[:, k, :],
                                     start=(k == 0), stop=(k == KT - 1))
                nc.scalar.activation(extras[:, n0:n0 + NW], pse[:], AF.Copy)
                for mi in range(4):
                    ps = ps_pool.tile([128, NW], f32, tag="ps", name=f"ps{blk}_{nn}_{mi}")
                    for k in range(KT):
                        nc.tensor.matmul(
                            ps[:], wc[:, k, 24 + mi * 128:24 + (mi + 1) * 128],
                            xt[:, k, :],
                            start=(k == 0), stop=(k == KT - 1))
                    nc.scalar.activation(ksb[mi][:, n0:n0 + NW], ps[:], AF.Copy)

            def scan_mi(blk, mi, extras, ksb):
                t0, t1 = blk * BLK, (blk + 1) * BLK
                exps = []
                for f in range(3):
                    et = exp_pool.tile([128, BLK], f32, tag=f"exp{f}",
                                       name=f"exp{f}_{blk}_{mi}")
                    for nn in range(NB):
                        n0 = nn * NW
                        pp = psel_pool.tile([128, NW], f32, tag="pp",
                                            name=f"pp{blk}_{mi}_{f}_{nn}")
                        nc.tensor.matmul(pp[:], selt[mi][f][:],
                                         extras[:, n0:n0 + NW],
                                         start=True, stop=True)
                        nc.scalar.activation(et[:, n0:n0 + NW], pp[:], AF.Copy)
                    exps.append(et)
                bexp, qexp, vexp = exps
                u = work_pool.tile([128, BLK], f32, tag="u", name=f"u{blk}_{mi}")
                nc.vector.tensor_mul(u[:], ksb[mi][:], vexp[:])
                c = work_pool.tile([128, BLK], f32, tag=f"c{mi}", bufs=2,
                                   name=f"c{blk}_{mi}")
                init = 0.0 if blk == 0 else prev_c[mi][:, BLK - 1:BLK]
                nc.vector.tensor_tensor_scan(c[:], bexp[:], u[:], init,
                                             ALU.mult, ALU.add)
                prev_c[mi] = c
                o = o_pool.tile([128, BLK], f32r, tag="o", name=f"o{blk}_{mi}")
                nc.vector.tensor_mul(o[:], c[:], qexp[:])
                nc.gpsimd.dma_start(out=o_out[mi * 128:(mi + 1) * 128, t0:t1], in_=o[:])

            prev = None
            for blk in range(S // BLK):
                extras = ext_pool.tile([24, BLK], f32r, tag="extras",
                                       name=f"extras{blk}")
                ksb = [ksb_pool.tile([128, BLK], f32, tag=f"k{mi}", name=f"ksb{mi}_{blk}")
                       for mi in range(4)]
                for nn in range(NB):
                    gemm_slab(blk, nn, extras, ksb)
                    if prev is not None:
                        pex, pksb = prev
                        scan_mi(blk - 1, 2 * nn, pex, pksb)
                        scan_mi(blk - 1, 2 * nn + 1, pex, pksb)
                nc.scalar.activation(extras[0:8, :], extras[0:8, :], AF.Sigmoid, bias=bb[:])
                prev = (extras, ksb)
            pex, pksb = prev
            for mi in range(4):
                scan_mi(S // BLK - 1, mi, pex, pksb)
    nc.compile()
    return nc


def _build_l2(use_gb=True):
    import concourse.bass as bass
    import concourse.mybir as mybir
    from concourse import tile, bacc

    f32, f32r = mybir.dt.float32, mybir.dt.float32r
    AF = mybir.ActivationFunctionType
    ALU = mybir.AluOpType

    nc = bacc.Bacc("TRN2", target_bir_lowering=False, debug=False, num_devices=8)
    oT = nc.dram_tensor("oT", [HID, TH], f32r, kind="ExternalInput")
    Wo = nc.dram_tensor("Wo", [HID, HID], f32r, kind="ExternalInput")
    xres = nc.dram_tensor("xres", [TH, HID], f32, kind="ExternalInput")
    bo = nc.dram_tensor("bo", [1, HID], f32, kind="ExternalInput")
    lng = nc.dram_tensor("lng", [1, HID], f32, kind="ExternalInput")
    lnb = nc.dram_tensor("lnb", [1, HID], f32, kind="ExternalInput")
    yout = nc.dram_tensor("yout", [TH, HID], f32, kind="ExternalOutput")

    KT = 8
    MT = TH // 128  # 16 token tiles
    NW = 512

    with tile.TileContext(nc) as tc:
        with tc.tile_pool(name="wo", bufs=1) as wo_pool, \
             tc.tile_pool(name="ot", bufs=3) as ot_pool, \
             tc.tile_pool(name="xr", bufs=3) as xr_pool, \
             tc.tile_pool(name="y", bufs=3) as y_pool, \
             tc.tile_pool(name="st", bufs=4) as st_pool, \
             tc.tile_pool(name="ps", bufs=6, space="PSUM") as ps_pool:

            wo = wo_pool.tile([128, KT, HID], f32r)
            for k in range(KT):
                nc.sync.dma_start(out=wo[:, k, :], in_=Wo[k * 128:(k + 1) * 128, :])
            g_rep = wo_pool.tile([128, HID], f32)
            nc.gpsimd.dma_start(out=g_rep[:], in_=bass.AP(lng, 0, [[0, 128], [1, HID]]))
            b_rep = wo_pool.tile([128, HID], f32)
            nc.gpsimd.dma_start(out=b_rep[:], in_=bass.AP(lnb, 0, [[0, 128], [1, HID]]))
            bo_f = wo_pool.tile([1, HID], f32)
            nc.gpsimd.dma_start(out=bo_f[:], in_=bo[:])
            bo_r = wo_pool.tile([1, HID], f32r)
            nc.vector.tensor_copy(bo_r[:], bo_f[:])
            ones_f = wo_pool.tile([1, 128], f32)
            nc.vector.memset(ones_f[:], 1.0)
            ones_r = wo_pool.tile([1, 128], f32r)
            nc.vector.tensor_copy(ones_r[:], ones_f[:])

            for m in range(MT):
                ot = ot_pool.tile([128, KT, 128], f32r)
                nc.sync.dma_start(
                    out=ot[:],
                    in_=oT.rearrange("(kt p) s -> p kt s", p=128)[:, :, m * 128:(m + 1) * 128])
                xr = xr_pool.tile([128, HID], f32)
                nc.gpsimd.dma_start(out=xr[:], in_=xres[m * 128:(m + 1) * 128, :])

                y = y_pool.tile([128, HID], f32)
                for n in range(2):
                    ps = ps_pool.tile([128, NW], f32)
                    for k in range(KT):
                        nc.tensor.matmul(ps[:], ot[:, k, :],
                                         wo[:, k, n * NW:(n + 1) * NW],
                                         start=(k == 0), stop=False)
                    nc.tensor.matmul(ps[:], ones_r[:], bo_r[:, n * NW:(n + 1) * NW],
                                     start=False, stop=True)
                    # y = psum + residual
                    nc.vector.tensor_add(y[:, n * NW:(n + 1) * NW], ps[:],
                                         xr[:, n * NW:(n + 1) * NW])

                stats = st_pool.tile([128, 8], f32, tag="stats")
                dump = y_pool.tile([128, HID], f32, tag="dump")
                nc.scalar.activation(dump[:], y[:], AF.Copy, accum_out=stats[:, 0:1])
                dump2 = y_pool.tile([128, HID], f32, tag="dump2")
                nc.scalar.activation(dump2[:], y[:], AF.Square, accum_out=stats[:, 1:2])
                # mu = s1/H ; var = s2/H - mu^2 ; rstd = 1/sqrt(var+eps)
                nc.vector.tensor_scalar_mul(stats[:, 2:3], stats[:, 0:1], 1.0 / HID)
                nc.vector.tensor_scalar_mul(stats[:, 3:4], stats[:, 1:2], 1.0 / HID)
                nc.vector.tensor_mul(stats[:, 4:5], stats[:, 2:3], stats[:, 2:3])
                nc.vector.tensor_scalar(stats[:, 5:6], stats[:, 3:4], stats[:, 4:5],
                                        EPS, ALU.subtract, ALU.add)
                nc.scalar.activation(stats[:, 6:7], stats[:, 5:6], AF.Sqrt)
                nc.vector.reciprocal(stats[:, 7:8], stats[:, 6:7])
                # z = (y - mu) * rstd ; out = z * g + b (g/b skipped when identity)
                z = y_pool.tile([128, HID], f32, tag="z")
                nc.vector.tensor_scalar(z[:], y[:], stats[:, 2:3], stats[:, 7:8],
                                        ALU.subtract, ALU.mult)
                if use_gb:
                    zg = y_pool.tile([128, HID], f32, tag="zg")
                    nc.vector.tensor_mul(zg[:], z[:], g_rep[:])
                    out_t = y_pool.tile([128, HID], f32, tag="out")
                    nc.vector.tensor_add(out_t[:], zg[:], b_rep[:])
                else:
                    out_t = z
                nc.scalar.dma_start(out=yout[m * 128:(m + 1) * 128, :], in_=out_t[:])

    nc.compile()
    return nc


_CACHE = {}


def _get_l1():
    if "l1" not in _CACHE:
        _CACHE["l1"] = _build_l1()
    return _CACHE["l1"]


def _get_l2(use_gb):
    key = ("l2", use_gb)
    if key not in _CACHE:
        _CACHE[key] = _build_l2(use_gb)
    return _CACHE[key]


LAST_EXEC_NS = None


def kernel(x, Wq, Wk, Wv, Wbeta, b_beta, Wo, b_o, ln_g, ln_b):
    import os
    from concourse.bass_utils import run_bass_kernel_spmd

    x = np.asarray(x, np.float32)
    Wq = np.asarray(Wq, np.float32); Wk = np.asarray(Wk, np.float32)
    Wv = np.asarray(Wv, np.float32); Wbeta = np.asarray(Wbeta, np.float32)
    b_beta = np.asarray(b_beta, np.float32); Wo = np.asarray(Wo, np.float32)
    b_o = np.asarray(b_o, np.float32)
    ln_g = np.asarray(ln_g, np.float32); ln_b = np.asarray(ln_b, np.float32)

    nc1 = _get_l1()
    use_gb = not (np.all(ln_g == 1.0) and np.all(ln_b == 0.0))
    nc2 = _get_l2(use_gb)
    trace = bool(os.environ.get("DELTANET_TRACE"))

    # column sums of Wq / Wv per head
    Wqs = Wq.reshape(HID, NH, D).sum(-1)   # (HID, NH)
    Wvs = Wv.reshape(HID, NH, D).sum(-1)

    xT = [np.ascontiguousarray(x[b].T) for b in range(B)]

    in1 = []
    for c in range(8):
        b, hg = c // 2, c % 2
        hs = slice(hg * HG, (hg + 1) * HG)
        Wcat = np.concatenate(
            [Wbeta[:, hs], Wqs[:, hs], Wvs[:, hs], Wk[:, hg * HG * D:(hg + 1) * HG * D]],
            axis=1)
        in1.append({
            "xT": xT[b],
            "Wcat": np.ascontiguousarray(Wcat),
            "bbeta": np.ascontiguousarray(b_beta[hs].reshape(8, 1)),
        })
    if trace:
        import shutil
        for dpath in ("/root/problem/work/trace_l1", "/root/problem/work/trace_l2"):
            shutil.rmtree(dpath, ignore_errors=True)
            os.makedirs(dpath, exist_ok=True)
    kw1 = dict(trace=True, tmpdir="/root/problem/work/trace_l1") if trace else dict(trace=False)
    r1 = run_bass_kernel_spmd(nc1, in1, list(range(8)), **kw1)

    # assemble oT per batch: rows = hid (head-major), cols = tokens
    oT = [np.concatenate([r1.results[2 * b]["o_out"], r1.results[2 * b + 1]["o_out"]],
                         axis=0) for b in range(B)]

    in2 = []
    for c in range(8):
        b, half = c // 2, c % 2
        ts = slice(half * TH, (half + 1) * TH)
        in2.append({
            "oT": np.ascontiguousarray(oT[b][:, ts]),
            "Wo": Wo,
            "xres": np.ascontiguousarray(x[b, ts, :]),
            "bo": b_o.reshape(1, HID),
            "lng": ln_g.reshape(1, HID),
            "lnb": ln_b.reshape(1, HID),
        })
    kw2 = dict(trace=True, tmpdir="/root/problem/work/trace_l2") if trace else dict(trace=False)
    r2 = run_bass_kernel_spmd(nc2, in2, list(range(8)), **kw2)

    global LAST_EXEC_NS
    LAST_EXEC_NS = (r1.exec_time_ns, r2.exec_time_ns)

    out = np.empty((B, S, HID), np.float32)
    for c in range(8):
        b, half = c // 2, c % 2
        out[b, half * TH:(half + 1) * TH, :] = r2.results[c]["yout"]
    return out



# revision 9
# speedup vs baseline: 1.3923x; 1.3923x over previous
"""DeltaNet layer kernel for 8 Trainium2 NeuronCores.

Math note: in the reference's _delta_scan, the update added to the (D,D)
state h is identical for every row and h0=0, so all rows of h stay equal
forever. The layer therefore reduces exactly to a per-(batch, head)
first-order scalar-decay recurrence on a D-vector:

    c_t = beta_t * c_{t-1} + k_t * vsum_t,   o_t = qsum_t * c_t

with vsum = sum_d v, qsum = sum_d q. qsum/vsum only need x @ col-sums of
Wq/Wv. The recurrence maps 1:1 onto the DVE tensor_tensor_scan
instruction (fp32 state, one lane per (head, d) pair, scan along
tokens), which is bit-exact vs a sequential fp32 loop.

Two SPMD launches on cores 0-7:
  L1: core (b, head-group of 8): GEMM1 (f32r) -> extras rows
      (zbeta/qsum/vsum) + k rows; sigmoid; DRAM-bounce replication of
      beta/vs/qs rows to 64 lanes per head; u = k*vs fused into psum
      evacuation; tensor_tensor_scan; o = qs*c -> f32r.
  L2: core (b, token-half): GEMM2 o @ Wo (+ b_o via a k=1 matmul row),
      residual add, LayerNorm (ACT accum_out stats + fused
      tensor_scalar), * ln_g + ln_b.
"""
import sys

sys.path.insert(0, "/opt/trn_rl_repo")

import numpy as np

B, S, HID, NH = 4, 4096, 1024, 16
D = HID // NH
EPS = 1e-5
HG = 8          # heads per L1 core
TH = S // 2     # tokens per L2 core

_PROGRAMS = None


def _build_l1():
    import concourse.bass as bass
    import concourse.mybir as mybir
    from concourse import tile, bacc

    f32, f32r = mybir.dt.float32, mybir.dt.float32r
    bf16 = mybir.dt.bfloat16
    AF = mybir.ActivationFunctionType
    ALU = mybir.AluOpType

    nc = bacc.Bacc("TRN2", target_bir_lowering=False, debug=False, num_devices=8)
    xT = nc.dram_tensor("xT", [HID, S], bf16, kind="ExternalInput")
    # Wcat columns: [zb(8) | qs(8) | vs(8) | k(512)]
    Wcat = nc.dram_tensor("Wcat", [HID, 536], bf16, kind="ExternalInput")
    bbeta = nc.dram_tensor("bbeta", [8, 1], f32, kind="ExternalInput")
    o_out = nc.dram_tensor("o_out", [HG * D, S], bf16, kind="ExternalOutput")

    KT = 8          # hid k-tiles
    NW = 512

    # selector matrices: exp[p, :] = fields_row(field, head(p)); head(p) = p // 64
    sels = []
    for mi in range(4):
        per_field = []
        for row0 in (0, 8, 16):  # beta/zb, qs, vs row blocks in extras
            m = np.zeros((24, 128), np.float32)
            for p in range(128):
                m[row0 + 2 * mi + p // 64, p] = 1.0
            per_field.append(m)
        sels.append(per_field)

    with tile.TileContext(nc) as tc:
        sel_dram = [[nc.inline_tensor(m, name=f"sel{mi}_{f}") for f, m in enumerate(row)]
                    for mi, row in enumerate(sels)]
        with tc.tile_pool(name="wc", bufs=1) as wc_pool, \
             tc.tile_pool(name="xt", bufs=2) as xt_pool, \
             tc.tile_pool(name="ksb", bufs=3) as ksb_pool, \
             tc.tile_pool(name="ext", bufs=2) as ext_pool, \
             tc.tile_pool(name="exp", bufs=3) as exp_pool, \
             tc.tile_pool(name="work", bufs=2) as work_pool, \
             tc.tile_pool(name="state", bufs=1) as state_pool, \
             tc.tile_pool(name="osb", bufs=3) as o_pool, \
             tc.tile_pool(name="ps", bufs=4, space="PSUM") as ps_pool, \
             tc.tile_pool(name="pse", bufs=1, space="PSUM") as pse_pool, \
             tc.tile_pool(name="psel", bufs=3, space="PSUM") as psel_pool:

            wc = wc_pool.tile([128, KT, 536], bf16)
            for k in range(KT):
                nc.sync.dma_start(out=wc[:, k, :], in_=Wcat[k * 128:(k + 1) * 128, :])
            bb = wc_pool.tile([8, 1], f32)
            nc.sync.dma_start(out=bb[:], in_=bbeta[:])
            selt = []
            for mi in range(4):
                per_field = []
                for f in range(3):
                    stf = wc_pool.tile([24, 128], f32, name=f"seltf{mi}_{f}")
                    nc.sync.dma_start(out=stf[:], in_=sel_dram[mi][f][:])
                    st = wc_pool.tile([24, 128], f32r, name=f"selt{mi}_{f}")
                    nc.vector.tensor_copy(st[:], stf[:])
                    per_field.append(st)
                selt.append(per_field)
            prev_c = [None] * 4

            BLK = 1024
            NB = BLK // NW

            def gemm_slab(blk, nn, extras, ksb):
                t0 = blk * BLK
                n0 = nn * NW
                xt = xt_pool.tile([128, KT, NW], bf16, tag="xt", name=f"xt{blk}_{nn}")
                nc.sync.dma_start(
                    out=xt[:],
                    in_=xT.rearrange("(kt p) s -> p kt s", p=128)[:, :, t0 + n0:t0 + n0 + NW])
                pse = pse_pool.tile([24, NW], f32, tag="pse", name=f"pse{blk}_{nn}")
                for k in range(KT):
                    nc.tensor.matmul(pse[:], wc[:, k, 0:24], xt[:, k, :],
                                     start=(k == 0), stop=(k == KT - 1))
                nc.scalar.activation(extras[:, n0:n0 + NW], pse[:], AF.Copy)
                for mi in range(4):
                    ps = ps_pool.tile([128, NW], f32, tag="ps", name=f"ps{blk}_{nn}_{mi}")
                    for k in range(KT):
                        nc.tensor.matmul(
                            ps[:], wc[:, k, 24 + mi * 128:24 + (mi + 1) * 128],
                            xt[:, k, :],
                            start=(k == 0), stop=(k == KT - 1))
                    nc.scalar.activation(ksb[mi][:, n0:n0 + NW], ps[:], AF.Copy)

            def scan_mi(blk, mi, extras, ksb):
                t0, t1 = blk * BLK, (blk + 1) * BLK
                exps = []
                for f in range(3):
                    et = exp_pool.tile([128, BLK], f32, tag=f"exp{f}",
                                       name=f"exp{f}_{blk}_{mi}")
                    for nn in range(NB):
                        n0 = nn * NW
                        pp = psel_pool.tile([128, NW], f32, tag="pp",
                                            name=f"pp{blk}_{mi}_{f}_{nn}")
                        nc.tensor.matmul(pp[:], selt[mi][f][:],
                                         extras[:, n0:n0 + NW],
                                         start=True, stop=True)
                        nc.scalar.activation(et[:, n0:n0 + NW], pp[:], AF.Copy)
                    exps.append(et)
                bexp, qexp, vexp = exps
                u = work_pool.tile([128, BLK], f32, tag="u", name=f"u{blk}_{mi}")
                nc.vector.tensor_mul(u[:], ksb[mi][:], vexp[:])
                c = work_pool.tile([128, BLK], f32, tag=f"c{mi}", bufs=2,
                                   name=f"c{blk}_{mi}")
                init = 0.0 if blk == 0 else prev_c[mi][:, BLK - 1:BLK]
                nc.vector.tensor_tensor_scan(c[:], bexp[:], u[:], init,
                                             ALU.mult, ALU.add)
                prev_c[mi] = c
                o = o_pool.tile([128, BLK], bf16, tag="o", name=f"o{blk}_{mi}")
                nc.vector.tensor_mul(o[:], c[:], qexp[:])
                nc.gpsimd.dma_start(out=o_out[mi * 128:(mi + 1) * 128, t0:t1], in_=o[:])

            prev = None
            for blk in range(S // BLK):
                extras = ext_pool.tile([24, BLK], f32r, tag="extras",
                                       name=f"extras{blk}")
                ksb = [ksb_pool.tile([128, BLK], f32, tag=f"k{mi}", name=f"ksb{mi}_{blk}")
                       for mi in range(4)]
                for nn in range(NB):
                    gemm_slab(blk, nn, extras, ksb)
                    if prev is not None:
                        pex, pksb = prev
                        scan_mi(blk - 1, 2 * nn, pex, pksb)
                        scan_mi(blk - 1, 2 * nn + 1, pex, pksb)
                nc.scalar.activation(extras[0:8, :], extras[0:8, :], AF.Sigmoid, bias=bb[:])
                prev = (extras, ksb)
            pex, pksb = prev
            for mi in range(4):
                scan_mi(S // BLK - 1, mi, pex, pksb)
    nc.compile()
    return nc


def _build_l2(use_gb=True):
    import concourse.bass as bass
    import concourse.mybir as mybir
    from concourse import tile, bacc

    f32, f32r = mybir.dt.float32, mybir.dt.float32r
    bf16 = mybir.dt.bfloat16
    AF = mybir.ActivationFunctionType
    ALU = mybir.AluOpType

    nc = bacc.Bacc("TRN2", target_bir_lowering=False, debug=False, num_devices=8)
    oT = nc.dram_tensor("oT", [HID, TH], bf16, kind="ExternalInput")
    Wo = nc.dram_tensor("Wo", [HID, HID], bf16, kind="ExternalInput")
    # residual with b_o pre-folded on host, bf16; cast to f32 by gpsimd DMA
    xres = nc.dram_tensor("xres", [TH, HID], bf16, kind="ExternalInput")
    lng = nc.dram_tensor("lng", [1, HID], f32, kind="ExternalInput")
    lnb = nc.dram_tensor("lnb", [1, HID], f32, kind="ExternalInput")
    yout = nc.dram_tensor("yout", [TH, HID], f32, kind="ExternalOutput")

    KT = 8
    MT = TH // 128  # 16 token tiles
    NW = 512

    with tile.TileContext(nc) as tc:
        with tc.tile_pool(name="wo", bufs=1) as wo_pool, \
             tc.tile_pool(name="ot", bufs=3) as ot_pool, \
             tc.tile_pool(name="xr", bufs=3) as xr_pool, \
             tc.tile_pool(name="y", bufs=3) as y_pool, \
             tc.tile_pool(name="st", bufs=4) as st_pool, \
             tc.tile_pool(name="ps", bufs=6, space="PSUM") as ps_pool:

            wo = wo_pool.tile([128, KT, HID], bf16)
            for k in range(KT):
                nc.sync.dma_start(out=wo[:, k, :], in_=Wo[k * 128:(k + 1) * 128, :])
            g_rep = wo_pool.tile([128, HID], f32)
            nc.gpsimd.dma_start(out=g_rep[:], in_=bass.AP(lng, 0, [[0, 128], [1, HID]]))
            b_rep = wo_pool.tile([128, HID], f32)
            nc.gpsimd.dma_start(out=b_rep[:], in_=bass.AP(lnb, 0, [[0, 128], [1, HID]]))

            for m in range(MT):
                ot = ot_pool.tile([128, KT, 128], bf16)
                nc.sync.dma_start(
                    out=ot[:],
                    in_=oT.rearrange("(kt p) s -> p kt s", p=128)[:, :, m * 128:(m + 1) * 128])
                xr = xr_pool.tile([128, HID], f32)
                nc.gpsimd.dma_start(out=xr[:], in_=xres[m * 128:(m + 1) * 128, :])

                y = y_pool.tile([128, HID], f32)
                for n in range(2):
                    ps = ps_pool.tile([128, NW], f32)
                    for k in range(KT):
                        nc.tensor.matmul(ps[:], ot[:, k, :],
                                         wo[:, k, n * NW:(n + 1) * NW],
                                         start=(k == 0), stop=(k == KT - 1))
                    # y = psum + residual (b_o folded into xres on host)
                    nc.vector.tensor_add(y[:, n * NW:(n + 1) * NW], ps[:],
                                         xr[:, n * NW:(n + 1) * NW])

                stats = st_pool.tile([128, 8], f32, tag="stats")
                dump = y_pool.tile([128, HID], f32, tag="dump")
                nc.scalar.activation(dump[:], y[:], AF.Copy, accum_out=stats[:, 0:1])
                dump2 = y_pool.tile([128, HID], f32, tag="dump2")
                nc.scalar.activation(dump2[:], y[:], AF.Square, accum_out=stats[:, 1:2])
                # mu = s1/H ; var = s2/H - mu^2 ; rstd = 1/sqrt(var+eps)
                nc.vector.tensor_scalar_mul(stats[:, 2:3], stats[:, 0:1], 1.0 / HID)
                nc.vector.tensor_scalar_mul(stats[:, 3:4], stats[:, 1:2], 1.0 / HID)
                nc.vector.tensor_mul(stats[:, 4:5], stats[:, 2:3], stats[:, 2:3])
                nc.vector.tensor_scalar(stats[:, 5:6], stats[:, 3:4], stats[:, 4:5],
                                        EPS, ALU.subtract, ALU.add)
                nc.scalar.activation(stats[:, 6:7], stats[:, 5:6], AF.Sqrt)
                nc.vector.reciprocal(stats[:, 7:8], stats[:, 6:7])
                # z = (y - mu) * rstd ; out = z * g + b (g/b skipped when identity)
                z = y_pool.tile([128, HID], f32, tag="z")
                nc.vector.tensor_scalar(z[:], y[:], stats[:, 2:3], stats[:, 7:8],
                                        ALU.subtract, ALU.mult)
                if use_gb:
                    zg = y_pool.tile([128, HID], f32, tag="zg")
                    nc.vector.tensor_mul(zg[:], z[:], g_rep[:])
                    out_t = y_pool.tile([128, HID], f32, tag="out")
                    nc.vector.tensor_add(out_t[:], zg[:], b_rep[:])
                else:
                    out_t = z
                nc.scalar.dma_start(out=yout[m * 128:(m + 1) * 128, :], in_=out_t[:])

    nc.compile()
    return nc


_CACHE = {}


def _get_l1():
    if "l1" not in _CACHE:
        _CACHE["l1"] = _build_l1()
    return _CACHE["l1"]


def _get_l2(use_gb):
    key = ("l2", use_gb)
    if key not in _CACHE:
        _CACHE[key] = _build_l2(use_gb)
    return _CACHE[key]


LAST_EXEC_NS = None


def kernel(x, Wq, Wk, Wv, Wbeta, b_beta, Wo, b_o, ln_g, ln_b):
    import os
    from concourse.bass_utils import run_bass_kernel_spmd

    x = np.asarray(x, np.float32)
    Wq = np.asarray(Wq, np.float32); Wk = np.asarray(Wk, np.float32)
    Wv = np.asarray(Wv, np.float32); Wbeta = np.asarray(Wbeta, np.float32)
    b_beta = np.asarray(b_beta, np.float32); Wo = np.asarray(Wo, np.float32)
    b_o = np.asarray(b_o, np.float32)
    ln_g = np.asarray(ln_g, np.float32); ln_b = np.asarray(ln_b, np.float32)

    nc1 = _get_l1()
    use_gb = not (np.all(ln_g == 1.0) and np.all(ln_b == 0.0))
    nc2 = _get_l2(use_gb)
    trace = bool(os.environ.get("DELTANET_TRACE"))

    import ml_dtypes
    bf16 = ml_dtypes.bfloat16

    # column sums of Wq / Wv per head
    Wqs = Wq.reshape(HID, NH, D).sum(-1)   # (HID, NH)
    Wvs = Wv.reshape(HID, NH, D).sum(-1)

    xT = [np.ascontiguousarray(x[b].T.astype(bf16)) for b in range(B)]

    in1 = []
    for c in range(8):
        b, hg = c // 2, c % 2
        hs = slice(hg * HG, (hg + 1) * HG)
        Wcat = np.concatenate(
            [Wbeta[:, hs], Wqs[:, hs], Wvs[:, hs], Wk[:, hg * HG * D:(hg + 1) * HG * D]],
            axis=1)
        in1.append({
            "xT": xT[b],
            "Wcat": np.ascontiguousarray(Wcat.astype(bf16)),
            "bbeta": np.ascontiguousarray(b_beta[hs].reshape(8, 1)),
        })
    if trace:
        import shutil
        for dpath in ("/root/problem/work/trace_l1", "/root/problem/work/trace_l2"):
            shutil.rmtree(dpath, ignore_errors=True)
            os.makedirs(dpath, exist_ok=True)
    kw1 = dict(trace=True, tmpdir="/root/problem/work/trace_l1") if trace else dict(trace=False)
    r1 = run_bass_kernel_spmd(nc1, in1, list(range(8)), **kw1)

    # assemble oT per batch: rows = hid (head-major), cols = tokens
    oT = [np.concatenate([r1.results[2 * b]["o_out"], r1.results[2 * b + 1]["o_out"]],
                         axis=0) for b in range(B)]

    Wo_b = np.ascontiguousarray(Wo.astype(bf16))
    in2 = []
    for c in range(8):
        b, half = c // 2, c % 2
        ts = slice(half * TH, (half + 1) * TH)
        in2.append({
            "oT": np.ascontiguousarray(oT[b][:, ts]),
            "Wo": Wo_b,
            "xres": np.ascontiguousarray((x[b, ts, :] + b_o).astype(bf16)),
            "lng": ln_g.reshape(1, HID),
            "lnb": ln_b.reshape(1, HID),
        })
    kw2 = dict(trace=True, tmpdir="/root/problem/work/trace_l2") if trace else dict(trace=False)
    r2 = run_bass_kernel_spmd(nc2, in2, list(range(8)), **kw2)

    global LAST_EXEC_NS
    LAST_EXEC_NS = (r1.exec_time_ns, r2.exec_time_ns)

    out = np.empty((B, S, HID), np.float32)
    for c in range(8):
        b, half = c // 2, c % 2
        out[b, half * TH:(half + 1) * TH, :] = r2.results[c]["yout"]
    return out



# revision 18
# speedup vs baseline: 1.5772x; 1.1328x over previous
"""DeltaNet layer kernel for 8 Trainium2 NeuronCores.

Math note: in the reference's _delta_scan, the update added to the (D,D)
state h is identical for every row and h0=0, so all rows of h stay equal
forever. The layer therefore reduces exactly to a per-(batch, head)
first-order scalar-decay recurrence on a D-vector:

    c_t = beta_t * c_{t-1} + k_t * vsum_t,   o_t = qsum_t * c_t

with vsum = sum_d v, qsum = sum_d q. qsum/vsum only need x @ col-sums of
Wq/Wv. The recurrence maps 1:1 onto the DVE tensor_tensor_scan
instruction (fp32 state, one lane per (head, d) pair, scan along
tokens), which is bit-exact vs a sequential fp32 loop.

Two SPMD launches on cores 0-7:
  L1: core (b, head-group of 8): GEMM1 (f32r) -> extras rows
      (zbeta/qsum/vsum) + k rows; sigmoid; DRAM-bounce replication of
      beta/vs/qs rows to 64 lanes per head; u = k*vs fused into psum
      evacuation; tensor_tensor_scan; o = qs*c -> f32r.
  L2: core (b, token-half): GEMM2 o @ Wo (+ b_o via a k=1 matmul row),
      residual add, LayerNorm (ACT accum_out stats + fused
      tensor_scalar), * ln_g + ln_b.
"""
import sys

sys.path.insert(0, "/opt/trn_rl_repo")

import numpy as np

B, S, HID, NH = 4, 4096, 1024, 16
D = HID // NH
EPS = 1e-5
HG = 8          # heads per L1 core
TH = S // 2     # tokens per L2 core

_PROGRAMS = None


def _build_l1():
    import concourse.bass as bass
    import concourse.mybir as mybir
    from concourse import tile, bacc

    f32, f32r = mybir.dt.float32, mybir.dt.float32r
    fp16 = mybir.dt.float16
    AF = mybir.ActivationFunctionType
    ALU = mybir.AluOpType

    nc = bacc.Bacc("TRN2", target_bir_lowering=False, debug=False, num_devices=8)
    xT = nc.dram_tensor("xT", [HID, S], fp16, kind="ExternalInput")
    # Wcat columns: [zb(8) | qs(8) | vs(8) | k(512)]
    Wcat = nc.dram_tensor("Wcat", [HID, 536], fp16, kind="ExternalInput")
    bbeta = nc.dram_tensor("bbeta", [8, 1], f32, kind="ExternalInput")
    o_out = nc.dram_tensor("o_out", [HG * D, S], fp16, kind="ExternalOutput")

    KT = 8          # hid k-tiles
    NW = 512

    # selector matrices: exp[p, :] = fields_row(field, head(p)); head(p) = p // 64
    sels = []
    for mi in range(4):
        per_field = []
        for row0 in (0, 8, 16):  # beta/zb, qs, vs row blocks in extras
            m = np.zeros((24, 128), np.float32)
            for p in range(128):
                m[row0 + 2 * mi + p // 64, p] = 1.0
            per_field.append(m)
        sels.append(per_field)

    NSLAB = S // NW  # 8 slabs of 512 tokens

    with tile.TileContext(nc) as tc:
        sel_dram = [[nc.inline_tensor(m, name=f"sel{mi}_{f}") for f, m in enumerate(row)]
                    for mi, row in enumerate(sels)]
        with tc.tile_pool(name="wc", bufs=1) as wc_pool, \
             tc.tile_pool(name="xt", bufs=3) as xt_pool, \
             tc.tile_pool(name="ext", bufs=2) as ext_pool, \
             tc.tile_pool(name="vex", bufs=3) as vex_pool, \
             tc.tile_pool(name="work", bufs=3) as work_pool, \
             tc.tile_pool(name="osb", bufs=3) as o_pool, \
             tc.tile_pool(name="psk", bufs=2, space="PSUM") as psk_pool, \
             tc.tile_pool(name="psb", bufs=2, space="PSUM") as psb_pool, \
             tc.tile_pool(name="psq", bufs=2, space="PSUM") as psq_pool, \
             tc.tile_pool(name="psv", bufs=1, space="PSUM") as psv_pool, \
             tc.tile_pool(name="pse", bufs=1, space="PSUM") as pse_pool:

            wc = wc_pool.tile([128, KT, 536], fp16)
            for k in range(KT):
                nc.sync.dma_start(out=wc[:, k, :], in_=Wcat[k * 128:(k + 1) * 128, :])
            bb = wc_pool.tile([8, 1], f32)
            nc.sync.dma_start(out=bb[:], in_=bbeta[:])
            selt = []
            for mi in range(4):
                per_field = []
                for f in range(3):
                    stf = wc_pool.tile([24, 128], f32, name=f"seltf{mi}_{f}")
                    nc.sync.dma_start(out=stf[:], in_=sel_dram[mi][f][:])
                    st = wc_pool.tile([24, 128], f32r, name=f"selt{mi}_{f}")
                    nc.vector.tensor_copy(st[:], stf[:])
                    per_field.append(st)
                selt.append(per_field)
            prev_c = [None] * 4

            for s in range(NSLAB):
                t0 = s * NW
                xt = xt_pool.tile([128, KT, NW], fp16, tag="xt", name=f"xt{s}")
                nc.sync.dma_start(
                    out=xt[:],
                    in_=xT.rearrange("(kt p) s -> p kt s", p=128)[:, :, t0:t0 + NW])
                # extras GEMM -> evac -> sigmoid on beta rows
                pse = pse_pool.tile([24, NW], f32, tag="pse", name=f"pse{s}")
                for k in range(KT):
                    nc.tensor.matmul(pse[:], wc[:, k, 0:24], xt[:, k, :],
                                     start=(k == 0), stop=(k == KT - 1))
                extras = ext_pool.tile([24, NW], f32r, tag="ext", name=f"ext{s}")
                nc.scalar.activation(extras[:], pse[:], AF.Copy)
                nc.scalar.activation(extras[0:8, :], extras[0:8, :], AF.Sigmoid,
                                     bias=bb[:])
                for mi in range(4):
                    # k GEMM into PSUM (consumed directly by u-mul)
                    kps = psk_pool.tile([128, NW], f32, tag="k", name=f"k{s}_{mi}")
                    for k in range(KT):
                        nc.tensor.matmul(
                            kps[:], wc[:, k, 24 + mi * 128:24 + (mi + 1) * 128],
                            xt[:, k, :],
                            start=(k == 0), stop=(k == KT - 1))
                    # selector expansions: beta/q stay in PSUM, v evac'd to SBUF
                    bps = psb_pool.tile([128, NW], f32, tag="b", name=f"b{s}_{mi}")
                    nc.tensor.matmul(bps[:], selt[mi][0][:], extras[:],
                                     start=True, stop=True)
                    qps = psq_pool.tile([128, NW], f32, tag="q", name=f"q{s}_{mi}")
                    nc.tensor.matmul(qps[:], selt[mi][1][:], extras[:],
                                     start=True, stop=True)
                    qex = vex_pool.tile([128, NW], fp16, tag="q", name=f"qx{s}_{mi}")
                    nc.scalar.activation(qex[:], qps[:], AF.Copy)
                    vps = psv_pool.tile([128, NW], f32, tag="v", name=f"v{s}_{mi}")
                    nc.tensor.matmul(vps[:], selt[mi][2][:], extras[:],
                                     start=True, stop=True)
                    vex = vex_pool.tile([128, NW], fp16, tag="v", name=f"vx{s}_{mi}")
                    nc.scalar.activation(vex[:], vps[:], AF.Copy)
                    # u = k * vexp ; c = scan(beta, u) ; o = qexp * c
                    u = work_pool.tile([128, NW], f32, tag="u", name=f"u{s}_{mi}")
                    nc.vector.tensor_mul(u[:], kps[:], vex[:])
                    c = work_pool.tile([128, NW], f32, tag=f"c{mi}", bufs=2,
                                       name=f"c{s}_{mi}")
                    init = 0.0 if s == 0 else prev_c[mi][:, NW - 1:NW]
                    nc.vector.tensor_tensor_scan(c[:], bps[:], u[:], init,
                                                 ALU.mult, ALU.add)
                    prev_c[mi] = c
                    o = o_pool.tile([128, NW], fp16, tag="o", name=f"o{s}_{mi}")
                    nc.gpsimd.tensor_mul(o[:], c[:], qex[:])
                    nc.gpsimd.dma_start(out=o_out[mi * 128:(mi + 1) * 128, t0:t0 + NW],
                                        in_=o[:])
    nc.compile()
    return nc


def _build_l2(use_gb=True):
    import concourse.bass as bass
    import concourse.mybir as mybir
    from concourse import tile, bacc

    f32, f32r = mybir.dt.float32, mybir.dt.float32r
    fp16 = mybir.dt.float16
    AF = mybir.ActivationFunctionType
    ALU = mybir.AluOpType

    nc = bacc.Bacc("TRN2", target_bir_lowering=False, debug=False, num_devices=8)
    oT = nc.dram_tensor("oT", [HID, TH], fp16, kind="ExternalInput")
    Wo = nc.dram_tensor("Wo", [HID, HID], fp16, kind="ExternalInput")
    # residual with b_o pre-folded on host, fp16; cast to f32 by gpsimd DMA
    xres = nc.dram_tensor("xres", [TH, HID], fp16, kind="ExternalInput")
    lng = nc.dram_tensor("lng", [1, HID], f32, kind="ExternalInput")
    lnb = nc.dram_tensor("lnb", [1, HID], f32, kind="ExternalInput")
    yout = nc.dram_tensor("yout", [TH, HID], fp16, kind="ExternalOutput")

    KT = 8
    MT = TH // 128  # 16 token tiles
    MG = 4          # token tiles per oT load
    NW = 512

    with tile.TileContext(nc) as tc:
        with tc.tile_pool(name="wo", bufs=1) as wo_pool, \
             tc.tile_pool(name="ot", bufs=3) as ot_pool, \
             tc.tile_pool(name="xr", bufs=3) as xr_pool, \
             tc.tile_pool(name="y", bufs=3) as y_pool, \
             tc.tile_pool(name="st", bufs=4) as st_pool, \
             tc.tile_pool(name="ps", bufs=6, space="PSUM") as ps_pool:

            wo = wo_pool.tile([128, KT, HID], fp16)
            for k in range(KT):
                nc.sync.dma_start(out=wo[:, k, :], in_=Wo[k * 128:(k + 1) * 128, :])
            g_rep = wo_pool.tile([128, HID], f32)
            nc.gpsimd.dma_start(out=g_rep[:], in_=bass.AP(lng, 0, [[0, 128], [1, HID]]))
            b_rep = wo_pool.tile([128, HID], f32)
            nc.gpsimd.dma_start(out=b_rep[:], in_=bass.AP(lnb, 0, [[0, 128], [1, HID]]))

            for m in range(MT):
                if m % MG == 0:
                    otg = ot_pool.tile([128, KT, MG * 128], fp16, tag="ot",
                                       name=f"ot{m}")
                    nc.sync.dma_start(
                        out=otg[:],
                        in_=oT.rearrange("(kt p) s -> p kt s", p=128)
                        [:, :, m * 128:(m + MG) * 128])
                ot = otg[:, :, (m % MG) * 128:(m % MG + 1) * 128]
                xr = xr_pool.tile([128, HID], f32)
                nc.gpsimd.dma_start(out=xr[:], in_=xres[m * 128:(m + 1) * 128, :])

                y = y_pool.tile([128, HID], f32)
                for n in range(2):
                    ps = ps_pool.tile([128, NW], f32)
                    for k in range(KT):
                        nc.tensor.matmul(ps[:], ot[:, k, :],
                                         wo[:, k, n * NW:(n + 1) * NW],
                                         start=(k == 0), stop=(k == KT - 1))
                    # y = psum + residual (b_o folded into xres on host)
                    nc.vector.tensor_add(y[:, n * NW:(n + 1) * NW], ps[:],
                                         xr[:, n * NW:(n + 1) * NW])

                stats = st_pool.tile([128, 8], f32, tag="stats")
                dump = y_pool.tile([128, HID], f32, tag="dump")
                nc.scalar.activation(dump[:], y[:], AF.Copy, accum_out=stats[:, 0:1])
                dump2 = y_pool.tile([128, HID], f32, tag="dump2")
                nc.scalar.activation(dump2[:], y[:], AF.Square, accum_out=stats[:, 1:2])
                # mu = s1/H ; var = s2/H - mu^2 ; rstd = 1/sqrt(var+eps)
                nc.vector.tensor_scalar_mul(stats[:, 2:3], stats[:, 0:1], 1.0 / HID)
                nc.vector.tensor_scalar_mul(stats[:, 3:4], stats[:, 1:2], 1.0 / HID)
                nc.vector.tensor_mul(stats[:, 4:5], stats[:, 2:3], stats[:, 2:3])
                nc.vector.tensor_scalar(stats[:, 5:6], stats[:, 3:4], stats[:, 4:5],
                                        EPS, ALU.subtract, ALU.add)
                nc.scalar.activation(stats[:, 6:7], stats[:, 5:6], AF.Sqrt)
                nc.vector.reciprocal(stats[:, 7:8], stats[:, 6:7])
                # z = (y - mu) * rstd ; out = z * g + b (g/b skipped when identity)
                if use_gb:
                    z = y_pool.tile([128, HID], f32, tag="z")
                    nc.vector.tensor_scalar(z[:], y[:], stats[:, 2:3], stats[:, 7:8],
                                            ALU.subtract, ALU.mult)
                    zg = y_pool.tile([128, HID], f32, tag="zg")
                    nc.vector.tensor_mul(zg[:], z[:], g_rep[:])
                    out_t = y_pool.tile([128, HID], fp16, tag="out")
                    nc.vector.tensor_add(out_t[:], zg[:], b_rep[:])
                else:
                    out_t = y_pool.tile([128, HID], fp16, tag="z")
                    nc.vector.tensor_scalar(out_t[:], y[:], stats[:, 2:3],
                                            stats[:, 7:8], ALU.subtract, ALU.mult)
                nc.scalar.dma_start(out=yout[m * 128:(m + 1) * 128, :], in_=out_t[:])

    nc.compile()
    return nc


_CACHE = {}


def _get_l1():
    if "l1" not in _CACHE:
        _CACHE["l1"] = _build_l1()
    return _CACHE["l1"]


def _get_l2(use_gb):
    key = ("l2", use_gb)
    if key not in _CACHE:
        _CACHE[key] = _build_l2(use_gb)
    return _CACHE[key]


LAST_EXEC_NS = None


def kernel(x, Wq, Wk, Wv, Wbeta, b_beta, Wo, b_o, ln_g, ln_b):
    import os
    from concourse.bass_utils import run_bass_kernel_spmd

    x = np.asarray(x, np.float32)
    Wq = np.asarray(Wq, np.float32); Wk = np.asarray(Wk, np.float32)
    Wv = np.asarray(Wv, np.float32); Wbeta = np.asarray(Wbeta, np.float32)
    b_beta = np.asarray(b_beta, np.float32); Wo = np.asarray(Wo, np.float32)
    b_o = np.asarray(b_o, np.float32)
    ln_g = np.asarray(ln_g, np.float32); ln_b = np.asarray(ln_b, np.float32)

    nc1 = _get_l1()
    use_gb = not (np.all(ln_g == 1.0) and np.all(ln_b == 0.0))
    nc2 = _get_l2(use_gb)
    trace = bool(os.environ.get("DELTANET_TRACE"))

    fp16 = np.float16
    # column sums of Wq / Wv per head
    Wqs = Wq.reshape(HID, NH, D).sum(-1)   # (HID, NH)
    Wvs = Wv.reshape(HID, NH, D).sum(-1)

    xT = [np.ascontiguousarray(x[b].T.astype(fp16)) for b in range(B)]

    in1 = []
    for c in range(8):
        b, hg = c // 2, c % 2
        hs = slice(hg * HG, (hg + 1) * HG)
        Wcat = np.concatenate(
            [Wbeta[:, hs], Wqs[:, hs], Wvs[:, hs], Wk[:, hg * HG * D:(hg + 1) * HG * D]],
            axis=1)
        in1.append({
            "xT": xT[b],
            "Wcat": np.ascontiguousarray(Wcat.astype(fp16)),
            "bbeta": np.ascontiguousarray(b_beta[hs].reshape(8, 1)),
        })
    if trace:
        import shutil
        for dpath in ("/root/problem/work/trace_l1", "/root/problem/work/trace_l2"):
            shutil.rmtree(dpath, ignore_errors=True)
            os.makedirs(dpath, exist_ok=True)
    kw1 = dict(trace=True, tmpdir="/root/problem/work/trace_l1") if trace else dict(trace=False)
    r1 = run_bass_kernel_spmd(nc1, in1, list(range(8)), **kw1)

    # assemble oT per batch: rows = hid (head-major), cols = tokens
    oT = [np.concatenate([r1.results[2 * b]["o_out"], r1.results[2 * b + 1]["o_out"]],
                         axis=0) for b in range(B)]

    Wo_b = np.ascontiguousarray(Wo.astype(fp16))
    in2 = []
    for c in range(8):
        b, half = c // 2, c % 2
        ts = slice(half * TH, (half + 1) * TH)
        in2.append({
            "oT": np.ascontiguousarray(oT[b][:, ts]),
            "Wo": Wo_b,
            "xres": np.ascontiguousarray((x[b, ts, :] + b_o).astype(fp16)),
            "lng": ln_g.reshape(1, HID),
            "lnb": ln_b.reshape(1, HID),
        })
    kw2 = dict(trace=True, tmpdir="/root/problem/work/trace_l2") if trace else dict(trace=False)
    r2 = run_bass_kernel_spmd(nc2, in2, list(range(8)), **kw2)

    global LAST_EXEC_NS
    LAST_EXEC_NS = (r1.exec_time_ns, r2.exec_time_ns)

    out = np.empty((B, S, HID), np.float32)
    for c in range(8):
        b, half = c // 2, c % 2
        out[b, half * TH:(half + 1) * TH, :] = r2.results[c]["yout"]
    return out

